# revision 1
# baseline (speedup 1.0000x reference)
"""Stereo correlation cost volume kernel for Trainium2 (8 NeuronCores).

  out[b, d, h, w] = mean_c( L[b,c,h,w] * R[b,c,h,w-d] )  for w >= d, else 0
  B=8, C=64, H=128, W=256, D=64.

Sharding: data-parallel over batch; core b handles batch b.

Per-core algorithm (per h row), with R pre-scaled by 1/C on the host:
  1. PE computes the needed band of the Gram G[u, w] = sum_c R[c,u]*L[c,w]
     as four 64x128 quadrant matmuls packed into one PSUM tile P[128, 256]:
       P[0:64,   0:128] = G[u=  0: 64, w=  0:128]   (M0)
       P[0:64, 128:256] = G[u= 64:128, w= 64:192]   (M1)
       P[64:128, 0:128] = G[u=128:192, w=128:256]   (M2)
       P[64:128,128:256] = G[u=192:256, w=128:256]  (M3)
     This stores, for every u-row, a 128..256-col window that covers the
     needed diagonal band {w = u+d, d in [0,64)} while keeping scratch
     rows at 256 contiguous fp16 elements (512 B -> full-rate DMA).
  2. The tile is cast fp32->fp16 (DVE/ACT alternating, 2 h per op) and
     DMA'd to a DRAM scratch as dense 256-el rows (row p of slot h at
     sbase + h*HSLOT + p*256).
  3. Skewed re-reads with stride 257 materialize T3a[u, d] = G[u, u+d]
     (u in [0,128)) and T3b for u in [128,256):
       t3a: (a,p,h,d) -> sbase + a*128 + p*257 + h*HSLOT + d
       t3b: (a,p,h,d) -> sbase + 16384 + a*192 + p*257 + h*HSLOT + d
     Reads that run past a row (w >= 256) land in later rows / the
     64-el zeroed slack; those columns are sliced off by the host.
  4. PE transposes T3 -> PT[d, u] (fp16 PSUM), DVE/ACT copy-cast to a
     fp32 SBUF tile s8, and one strided DMA per group writes
     out[d, h, w=u+d] (partition stride H*WP+1, 1 KB runs).
The output DRAM tensor is padded to WP=320 columns so skew/write spill
for w >= 256 is harmless; the host slices w < 256.  The runner
pre-zeros output buffers, so the w < d triangle stays zero.

DMA issuers: SP carries input loads + skew reads, ACT the scratch
writes, Pool (SWDGE) the output writes - spreading queue occupancy and
keeping HWDGE under ~3 instructions per group.
"""

import os
import sys

import numpy as np

sys.path.insert(0, "/opt/trn_rl_repo")

import concourse.bass as bass  # noqa: E402
import concourse.bacc as bacc  # noqa: E402
import concourse.mybir as mybir  # noqa: E402
from concourse.bass import AP  # noqa: E402
from concourse.bass_utils import run_bass_kernel_spmd  # noqa: E402
from concourse.masks import make_identity  # noqa: E402
from concourse.tile import TileContext  # noqa: E402

B, C, H, W = 8, 64, 128, 256
D = 64
WP = 320  # padded output width
NH = int(os.environ.get("KN_NH", 8))  # h rows per group
NG = H // NH  # 16 groups
F32 = mybir.dt.float32
F16 = mybir.dt.float16

ROW = 256  # scratch row elements per h-slot row
HSLOT = 128 * ROW + 64  # per-h slot: 128 rows + 64-el slack for b3 spill
SCR_SIZE = (NG // 4) * NH * HSLOT  # one slot per (group//4, h)

_CACHE = {}

# tuning knobs (sim experiments)
_K = lambda name, dflt: int(os.environ.get(name, dflt))
KN_WSPLIT = _K("KN_WSPLIT", 0)   # split scratch write into A/B partition halves
KN_OSPLIT = _K("KN_OSPLIT", 1)   # split out DMA into 2 h-halves
KN_BUF_G = _K("KN_BUF_G", 3)
KN_BUF_T3 = _K("KN_BUF_T3", 3)
KN_BUF_S8 = _K("KN_BUF_S8", 4)
KN_BUF_PG = _K("KN_BUF_PG", 4)
KN_BUF_PT = _K("KN_BUF_PT", 4)
KN_OUT_ENG = os.environ.get("KN_OUT_ENG", "gpsimd")
KN_SKEW_ENG = os.environ.get("KN_SKEW_ENG", "sync")


def build(reps=1):
    """reps>1 wraps the whole computation in a hardware loop - used only
    by bench.py to measure per-iteration HW time via chain-length slope."""
    import contextlib

    nc = bacc.Bacc()
    lr_dram = nc.dram_tensor("lr", [C, H, 2, W], F16, kind="ExternalInput")
    out_dram = nc.dram_tensor("out", [D, H, WP], F32, kind="ExternalOutput")
    scr = [
        nc.dram_tensor(f"scratch{i}", [SCR_SIZE], F16, kind="Internal")
        for i in range(4)
    ]

    with TileContext(nc) as tc:
        with (
            tc.tile_pool(name="const", bufs=1) as pconst,
            tc.tile_pool(name="inp", bufs=_K("KN_BUF_IN", 2)) as pin,
            tc.tile_pool(name="gband", bufs=KN_BUF_G) as pg,
            tc.tile_pool(name="skew", bufs=KN_BUF_T3) as pt3,
            tc.tile_pool(name="outs", bufs=KN_BUF_S8) as ps8,
            tc.tile_pool(name="psG", bufs=KN_BUF_PG, space="PSUM") as ppg,
            tc.tile_pool(name="psT", bufs=KN_BUF_PT, space="PSUM") as ppt,
        ):
            def load(g):
                lr8 = pin.tile([C, NH * 2 * W], F16, tag="lr8")
                lr8v = lr8.rearrange("p (h t w) -> p h t w", h=NH, t=2)
                nc.sync.dma_start(
                    out=lr8v, in_=lr_dram[:, g * NH : g * NH + NH, :, :]
                )
                return lr8v

            first_lr = load(0) if reps == 1 else None

            ident = pconst.tile([128, 128], F16)
            make_identity(nc, ident)
            zeros = pconst.tile([64, 64], F16)
            nc.gpsimd.memset(zeros, 0.0)
            # zero the per-slot slack so skew-read spill never reads uninit
            for i in range(4):
                nc.scalar.dma_start(
                    out=AP(
                        scr[i],
                        128 * ROW,
                        [[HSLOT, (NG // 4) * NH], [1, 64]],
                    ),
                    in_=zeros[0 : (NG // 4) * NH, :],
                )
            # warmup: absorb the gpsimd ident-write wait on PE once
            scrap0 = ppg.tile([64, 64], F16, tag="P")
            nc.tensor.transpose(
                scrap0[0:1, :], ident[0:64, 0:1], ident[0:64, 0:64]
            )

            rep_ctx = (
        contextlib.nullcontext() if reps == 1 else tc.For_i(0, reps)
            )
            rep_stack = contextlib.ExitStack()
            rep_stack.enter_context(rep_ctx)

            skew_q = []  # groups whose gram is written but not yet skew-read
            pending = None  # (t3av, t3bv, g) ready for transpose+out

            def do_skew(g, hsplit=1):
                st = scr[g % 4]
                sbase = (g // 4) * NH * HSLOT
                t3a = pt3.tile([128, NH * 64], F16, tag="t3a")
                t3b = pt3.tile([128, NH * 64], F16, tag="t3b")
                t3av = t3a.rearrange("p (h d) -> p h d", h=NH)
                t3bv = t3b.rearrange("p (h d) -> p h d", h=NH)
                skew_eng = getattr(nc, KN_SKEW_ENG)
                nhh = NH // hsplit
                for hs in range(hsplit):
                    hb = hs * nhh
                    sb = sbase + hb * HSLOT
                    for half, off in ((0, 0), (1, 128)):
                        skew_eng.dma_start(
                            out=t3av[
                                64 * half : 64 * half + 64,
                                hb : hb + nhh,
                                :,
                            ],
                            in_=AP(
                                st,
                                sb + off,
                                [[257, 64], [HSLOT, nhh], [1, 64]],
                            ),
                        )
                    for half, off in ((0, 0), (1, 192)):
                        skew_eng.dma_start(
                            out=t3bv[
                                64 * half : 64 * half + 64,
                                hb : hb + nhh,
                                :,
                            ],
                            in_=AP(
                                st,
                                sb + 64 * ROW + off,
                                [[257, 64], [HSLOT, nhh], [1, 64]],
                            ),
                        )
                return (t3av, t3bv, g)

            def consume(pend, osplit=0):
                """Transposes + out-copies + out-DMA for a finished group."""
                t3av, t3bv, pg_ = pend
                s8 = ps8.tile([64, NH * 256], F32, tag="s8")
                s8v = s8.rearrange("p (hp c) -> p hp c", hp=NH // 2)
                for hp in range(NH // 2):
                    pt = ppt.tile([64, 512], F16, tag="pt")
                    for k in range(2):
                        hh = 2 * hp + k
                        nc.tensor.transpose(
                            pt[:, 256 * k : 256 * k + 128],
                            t3av[:, hh, :],
                            ident,
                        )
                        nc.tensor.transpose(
                            pt[:, 256 * k + 128 : 256 * k + 256],
                            t3bv[:, hh, :],
                            ident,
                        )
                    if hp % 2 == 0:
                        nc.vector.tensor_copy(s8v[:, hp, :], pt)
                    else:
                        nc.scalar.copy(s8v[:, hp, :], pt)
                # out write on Pool/SWDGE: out[d, h, w=u+d]
                out_eng = getattr(nc, KN_OUT_ENG)
                nsp = osplit if osplit else (2 if KN_OSPLIT else 1)
                hh2 = NH // nsp
                for j in range(nsp):
                    out_eng.dma_start(
                        out=AP(
                            out_dram,
                            (pg_ * NH + j * hh2) * WP,
                            [[H * WP + 1, 64], [WP, hh2], [1, 256]],
                        ),
                        in_=s8v[
                            :, j * (hh2 // 2) : (j + 1) * (hh2 // 2), :
                        ],
                    )

            for g in range(NG):
                st = scr[g % 4]
                sbase = (g // 4) * NH * HSLOT

                # stage 1: input load [SP]
                lr8v = (
                    first_lr
                    if g == 0 and first_lr is not None
                    else load(g)
                )

                # stage 2: consume previous group (PE/DVE/ACT first)
                if pending is not None:
                    consume(pending)

                # stage 3: produce this group's Gram band
                g8 = pg.tile([128, NH * ROW], F16, tag="g8")
                g8v = g8.rearrange("p (h c) -> p h c", h=NH)
                g8p = g8.rearrange("p (hp c) -> p hp c", hp=NH // 2)
                for hp in range(NH // 2):
                    P = ppg.tile([128, 512], F32, tag="P")
                    for k in range(2):
                        hh = 2 * hp + k
                        co = 256 * k
                        Rv = lr8v[:, hh, 1, :]
                        Lv = lr8v[:, hh, 0, :]
                        nc.tensor.matmul(
                            P[0:64, co : co + 128],
                            lhsT=Rv[:, 0:64],
                            rhs=Lv[:, 0:128],
                        )
                        nc.tensor.matmul(
                            P[0:64, co + 128 : co + 256],
                            lhsT=Rv[:, 64:128],
                            rhs=Lv[:, 64:192],
                        )
                        nc.tensor.matmul(
                            P[64:128, co : co + 128],
                            lhsT=Rv[:, 128:192],
                            rhs=Lv[:, 128:256],
                        )
                        nc.tensor.matmul(
                            P[64:128, co + 128 : co + 256],
                            lhsT=Rv[:, 192:256],
                            rhs=Lv[:, 128:256],
                        )
                    if hp % 2 == 0:
                        nc.scalar.copy(g8p[:, hp, :], P)
                    else:
                        nc.vector.tensor_copy(g8p[:, hp, :], P)

                # stage 4: Gram band -> scratch [ACT]
                if KN_WSPLIT:
                    nc.scalar.dma_start(
                        out=AP(
                            st, sbase, [[ROW, 64], [HSLOT, NH], [1, ROW]]
                        ),
                        in_=g8v[0:64, :, :],
                    )
                    nc.scalar.dma_start(
                        out=AP(
                            st,
                            sbase + 64 * ROW,
                            [[ROW, 64], [HSLOT, NH], [1, ROW]],
                        ),
                        in_=g8v[64:128, :, :],
                    )
                else:
                    nc.scalar.dma_start(
                        out=AP(st, sbase, [[ROW, 128], [HSLOT, NH], [1, ROW]]),
                        in_=g8v,
                    )
                # stage 5: skewed re-read [SP]
                pending = do_skew(g)


            consume(pending)
            rep_stack.close()
    nc.finalize()
    return nc


def kernel(left_feature, right_feature, max_disp):
    assert int(max_disp) == D
    left = np.asarray(left_feature, dtype=np.float32)
    right = np.asarray(right_feature, dtype=np.float32)
    assert left.shape == (B, C, H, W) and right.shape == (B, C, H, W)

    if "nc" not in _CACHE:
        _CACHE["nc"] = build()
    nc = _CACHE["nc"]

    # fold the mean's 1/C into R on the host (exact fp16 scale by 2^-6)
    right_s = right * (1.0 / C)
    in_maps = []
    for b in range(B):
        lr = np.ascontiguousarray(
            np.stack([left[b], right_s[b]], axis=2).astype(np.float16)
        )  # [C, H, 2, W]
        in_maps.append({"lr": lr})
    res = run_bass_kernel_spmd(nc, in_maps, list(range(B)))
    _CACHE["last_results"] = res
    out = np.stack([res.results[b]["out"][:, :, :W] for b in range(B)], axis=0)
    return out.astype(np.float32)

